# revision 1
# baseline (speedup 1.0000x reference)
"""Trainium2 Bass kernel for nn_AttentionModule_50002009260608 — fp8 redesign.

B=16, C=512, H=W=24 (HW=576), TF=512, NH=8, CPH=64. Data-parallel: 2 batch/core.

All heavy matmuls in fp8e4m3 with DoubleRow perf mode (0.5 cyc/row). Weights
scaled x32 host-side; scale ledger (powers of 2, exact):
  Q8,K8,vl8 = 2^5 * true; VT = 2^4 * V; es = exp(l*SCALE) (x1, via exp-scale
  2^-10); ones col = 0.125 so sums-col = sum(es)/8, r = 8/sum; outTn/outall =
  2^7 * out_self; crossn_s = 2^6 * crossn; crossout8 = 2^5 * crossout; fin
  main psum = 2^12 * Wr@out_self, cross col = 2^10 * Wr@crossout.
Host applies 2^-12 / 2^-10 and bias = Wr_b + 2*Wr@(Wv@Wm2@t) (tvec+cross
value-bias folded out of the device pipeline algebraically).

AV is computed transposed (out[n_query, c]) so softmax normalization is a
per-partition tensor_scalar; a PE transpose (fp8 identity) restores [c, n].
exp runs as [128,4,288] activations spanning a 4-bank psum tile (3/head).
QK uses DoubleRow over the 64-contraction via a DRAM round-trip relayout of
Q8/K8 into [32, hi, k, n] (c = 128g + 64hi + 32k + lane, g=h//2, hi=h%2).
"""

import numpy as np
from contextlib import ExitStack

import concourse.bacc as bacc
import concourse.bass as bass
from concourse.bass import broadcast_tensor_aps
import concourse.tile as tile
import concourse.mybir as mybir
from concourse import masks
from concourse.bass_utils import run_bass_kernel_spmd

B, C, HW, TF, NH, CPH = 16, 512, 576, 512, 8, 64
NCORES, BPC = 8, B // 8
SCALE = 1.0 / 8.0
F32, BF16, F8 = mybir.dt.float32, mybir.dt.bfloat16, mybir.dt.float8e4
AF = mybir.ActivationFunctionType
OP = mybir.AluOpType
DR = mybir.MatmulPerfMode.DoubleRow
PD = 128
NCC = 4                                   # 128-channel chunks
MT = [(0, 128), (128, 128), (256, 128), (384, 128), (512, 64)]  # m tiles
NCK = [(96 * i, 96) for i in range(6)]     # AV n-chunks (uniform -> no pad rows)
NHALF = [(0, 288), (288, 288)]            # conv psum halves
FINH = [(0, 288), (288, 289)]             # final halves (incl cross col 576)
ES_W = HW                                 # es cols
OAW = HW + 1                              # outall cols (incl crossout col)

# fallback switches (flip if a mechanism fails on sim/hw)
USE_QK_DR = True        # DoubleRow QK via dram-roundtrip relayout
USE_PAIR_EXP = True     # [128,4,288] exp instrs spanning 4-bank psum
C8 = 1.0625             # half-ulp pre-compensation: fp8 casts truncate on HW
CB = 1.001953125        # same for bf16 casts
LNC8 = float(np.log(1.0625))


def _body(ctx: ExitStack, tc, d):
    nc = tc.nc

    wt = ctx.enter_context(tc.tile_pool(name="wt", bufs=1))
    act = ctx.enter_context(tc.tile_pool(name="act", bufs=1))
    esp = ctx.enter_context(tc.tile_pool(name="esp", bufs=1))
    pqk = ctx.enter_context(tc.tile_pool(name="pqk", bufs=1, space="PSUM"))
    pcv = ctx.enter_context(tc.tile_pool(name="pcv", bufs=1, space="PSUM"))
    pav = ctx.enter_context(tc.tile_pool(name="pav", bufs=1, space="PSUM"))

    # ---- weights + identities -------------------------------------------
    W = {}
    for wn in ("Wq8", "Wk8", "Wm18", "Wv8", "Wr8"):
        wtile = wt.tile([PD, NCC, C], F8, name=f"{wn}_t")
        nc.sync.dma_start(wtile[:], d[wn].rearrange("(cc p) o -> p cc o", p=PD))
        W[wn] = wtile
    identb = wt.tile([PD, PD], BF16, name="identb")
    masks.make_identity(nc, identb[:])
    lnc8 = wt.tile([PD, 1], F32, name="lnc8")
    nc.vector.memset(lnc8[:], LNC8)

    st = {}

    def emit_loads(b):
        x8 = act.tile([PD, NCC, HW], F8, name=f"x8_{b}", tag="x8", bufs=2)
        nc.sync.dma_start(x8[:], d["x8"][b].rearrange("(cc p) n -> p cc n", p=PD))
        tm8 = act.tile([PD, NCC, NH], F8, name=f"tm8_{b}", tag="tm8", bufs=2)
        nc.sync.dma_start(tm8[:], d["tm8"][b].rearrange("(cc p) h -> p cc h", p=PD))
        st[b] = {"x8": x8, "tm8": tm8}

    def conv_dr(b, Wn, rhs, dst, tag):
        """dst[128, NCC(ot), HW] (fp8) = fp8(Wn^T @ rhs); DR over cc pairs."""
        for ot2 in range(2):
            p = pqk.tile([PD, 4, 512], F32, tag="cw", bufs=1,
                         name=f"p_{tag}{b}_{ot2}")
            for oti in range(2):
                ot = ot2 * 2 + oti
                for hi, (n0, nsz) in enumerate(NHALF):
                    o = p[:, oti * 2 + hi, 0:nsz]
                    for cp in range(2):
                        nc.tensor.matmul(
                            o, Wn[:, 2 * cp:2 * cp + 2, ot * PD:(ot + 1) * PD],
                            rhs[:, 2 * cp:2 * cp + 2, n0:n0 + nsz],
                            start=(cp == 0), stop=(cp == 1), perf_mode=DR)
                dv = dst[:, ot, :].rearrange("p (a n) -> p a n", a=2)
                nc.vector.tensor_scalar_mul(dv, p[:, oti * 2:oti * 2 + 2, 0:288],
                                            C8)

    def emit_qk_conv(b, which):
        s = st[b]
        t8 = act.tile([PD, NCC, HW], F8, name=f"{which}8_{b}", tag=f"{which}8",
                      bufs=2)
        conv_dr(b, W[f"W{which}8"], s["x8"], t8, f"c{which}")
        s[f"{which}8"] = t8
        # dram roundtrip -> DoubleRow layout [32*g+lane, hi, k, n]
        if USE_QK_DR:
            scr = d[f"scr_{which}"][b]
            nc.sync.dma_start(scr.rearrange("(cc p) n -> p cc n", p=PD), t8[:])
            tdr = act.tile([PD, 2, 2, HW], F8, name=f"{which}dr_{b}",
                           tag=f"{which}dr", bufs=2)
            srcv = scr.rearrange("(g hi k lane) n -> g lane (hi k) n",
                                 g=4, hi=2, k=2, lane=32)
            for gg in range(4):
                nc.sync.dma_start(
                    tdr[32 * gg:32 * gg + 32].rearrange(
                        "p a b n -> p (a b) n"),
                    srcv[gg])
            s[f"{which}dr"] = tdr

    def emit_vl_v(b):
        s = st[b]
        vl8 = act.tile([PD, NCC, HW], F8, name=f"vl8_{b}", tag="vl8", bufs=2)
        conv_dr(b, W["Wm18"], s["x8"], vl8, "cvl")
        VT1 = act.tile([PD, 5, NH * 65], F8, name=f"VT1_{b}", tag="vt1", bufs=2)
        if b < 2:  # ones cols written once per physical buffer (2 bufs)
            ones_ap = VT1[:].rearrange("p mi (h c) -> p mi h c", h=NH)[:, :, :, 64:65]
            nc.gpsimd.memset(ones_ap, 0.125)
        for mi, (m0, msz) in enumerate(MT):
            p = pcv.tile([PD, 2, 512], F32, tag="cv", bufs=1, name=f"p_v{b}_{mi}")
            o = p[0:msz, 0, :]
            for cp in range(2):
                nc.tensor.matmul(o, vl8[:, 2 * cp:2 * cp + 2, m0:m0 + msz],
                                 W["Wv8"][:, 2 * cp:2 * cp + 2, :],
                                 start=(cp == 0), stop=(cp == 1), perf_mode=DR)
            vdst = VT1[0:msz, mi, :].rearrange("p (h c) -> p h c", h=NH)[:, :, 0:64]
            nc.vector.tensor_scalar_mul(
                vdst, p[0:msz, 0, :].rearrange("p (h c) -> p h c", h=NH),
                2.0 ** -6 * C8)
        s["VT1"] = VT1

    def emit_cross(b):
        s = st[b]
        p = pcv.tile([NH, 2, 512], F32, tag="cv", bufs=1, name=f"p_cr{b}")
        for hi, (n0, nsz) in enumerate(NHALF):
            o = p[:, hi, 0:nsz]
            for cc in range(NCC):
                nc.tensor.matmul(o, s["tm8"][:, cc, :],
                                 s["x8"][:, cc, n0:n0 + nsz],
                                 start=(cc == 0), stop=(cc == NCC - 1))
        crosse = act.tile([NH, HW], F32, name=f"crosse{b}", tag="crosse")
        csum = act.tile([NH, 1], F32, name=f"csum{b}", tag="csum")
        nc.scalar.activation(
            crosse[:].rearrange("p (a n) -> p a n", a=2), p[:, :, 0:288],
            AF.Exp, scale=SCALE / 32.0, accum_out=csum[:])
        crec = act.tile([NH, 1], F32, name=f"crec{b}", tag="crec")
        nc.vector.reciprocal(crec[:], csum[:])
        crossn_s = act.tile([NH, HW], BF16, name=f"crossn{b}", tag="crossn")
        nc.vector.tensor_scalar(crossn_s[:], crosse[:], crec[:], 64.0 * CB,
                                op0=OP.mult, op1=OP.mult)
        # transpose [8, m] -> [m, 8] then fp8 copy
        crossTm = act.tile([PD, 5, NH], F8, name=f"crossTm{b}", tag="crossTm")
        for mi, (m0, msz) in enumerate(MT):
            pt = pav.tile([PD, 512], BF16, tag="av", bufs=2,
                          name=f"p_ct{b}_{mi}")
            nc.tensor.transpose(pt[0:msz, 0:NH], crossn_s[:, m0:m0 + msz],
                                identb[0:NH, 0:NH])
            nc.vector.tensor_scalar_mul(crossTm[0:msz, mi, :], pt[0:msz, 0:NH], C8)
        s["crossTm"] = crossTm
        s["outall"] = act.tile([PD, NCC, OAW], F8, name=f"oa{b}", tag="oa",
                               bufs=2)

    def _qk_ops(b, h):
        s = st[b]
        g, hi = h // 2, h % 2

        def f(m0, msz, n0, nsz):
            if USE_QK_DR:
                kdr, qdr = s["kdr"], s["qdr"]
                return (kdr[32 * g:32 * g + 32, hi, :, m0:m0 + msz],
                        qdr[32 * g:32 * g + 32, hi, :, n0:n0 + nsz], DR,
                        (32 * g, 0))
            p0 = 64 * (h % 2)
            return (s["k8"][p0:p0 + CPH, h // 2, m0:m0 + msz],
                    s["q8"][p0:p0 + CPH, h // 2, n0:n0 + nsz], None, (p0, 0))
        return f

    def emit_head_qk_a(b, h):
        s = st[b]
        es = esp.tile([PD, 5, ES_W], F8, name=f"es{b}_{h}", tag="es", bufs=3)
        s[f"es{h}"] = es
        qk = _qk_ops(b, h)
        m0l, mszl = MT[4]
        pl = pcv.tile([64, 2, 512], F32, tag="cv", bufs=1, name=f"p_ql{b}_{h}")
        for hi2, (n0, nsz) in enumerate(NHALF):
            lhs, rhs, pm, tp = qk(m0l, mszl, n0, nsz)
            nc.tensor.matmul(pl[:, hi2, 0:nsz], lhs, rhs, start=True, stop=True,
                             perf_mode=pm, tile_position=tp,
                             skip_group_check=True)
        nc.scalar.activation(
            es[0:64, 4, :].rearrange("p (a n) -> p a n", a=2), pl[:, :, 0:288],
            AF.Exp, scale=SCALE / 1024.0 / (C8 * C8), bias=lnc8[0:64, :])
        _qk_pair(b, h, 0)

    def _qk_pair(b, h, pi):
        s = st[b]
        es = s[f"es{h}"]
        qk = _qk_ops(b, h)
        pp = pqk.tile([PD, 4, 512], F32, tag="cw", bufs=1,
                      name=f"p_qk{b}_{h}_{pi}")
        for mj in range(2):
            m0, msz = MT[2 * pi + mj]
            for hi2, (n0, nsz) in enumerate(NHALF):
                lhs, rhs, pm, tp = qk(m0, msz, n0, nsz)
                nc.tensor.matmul(pp[:, mj * 2 + hi2, 0:nsz], lhs, rhs,
                                 start=True, stop=True, perf_mode=pm,
                                 tile_position=tp, skip_group_check=True)
        nc.scalar.activation(
            es[:, 2 * pi:2 * pi + 2, :].rearrange("p mi (a n) -> p mi a n", a=2),
            pp[:, :, 0:288].rearrange("p (mi a) n -> p mi a n", mi=2),
            AF.Exp, scale=SCALE / 1024.0 / (C8 * C8), bias=lnc8[:, :])

    def emit_head_qk_b(b, h):
        _qk_pair(b, h, 1)

    def emit_head_av_a(b, h):
        s = st[b]
        es = s[f"es{h}"]
        pavt = pav.tile([PD, 6, 66], F32, tag="av", bufs=2, name=f"p_av{b}_{h}")
        s[f"pavt{h}"] = pavt
        vs = s["VT1"][:].rearrange("p mi (h c) -> p mi h c", h=NH)
        for pi in range(2):
            for ci, (n0, nsz) in enumerate(NCK):
                nc.tensor.matmul(
                    pavt[0:nsz, ci, 0:65], es[:, 2 * pi:2 * pi + 2, n0:n0 + nsz],
                    vs[:, 2 * pi:2 * pi + 2, h, :], start=(pi == 0), stop=False,
                    perf_mode=DR, skip_group_check=True)

    def emit_head_av_b(b, h):
        s = st[b]
        es = s[f"es{h}"]
        pavt = s[f"pavt{h}"]
        vs = s["VT1"][:].rearrange("p mi (h c) -> p mi h c", h=NH)
        for ci, (n0, nsz) in enumerate(NCK):
            nc.tensor.matmul(pavt[0:nsz, ci, 0:65], es[0:64, 4, n0:n0 + nsz],
                             vs[0:64, 4, h, :], start=False, stop=True,
                             skip_group_check=True)
        p0 = 64 * (h % 2)
        cps = pavt[p0:p0 + 64, 0, 65:66]
        for mi, (m0, msz) in enumerate(MT):
            nc.tensor.matmul(
                cps, vs[0:msz, mi, h, 0:64], s["crossTm"][0:msz, mi, h:h + 1],
                start=(mi == 0), stop=(mi == len(MT) - 1),
                skip_group_check=True)
        nc.vector.tensor_scalar_mul(
            s["outall"][p0:p0 + 64, h // 2, HW:HW + 1], cps, 2.0 ** -5 * C8)
        r = act.tile([PD, 6, 1], F32, name=f"r{b}_{h}", tag="r", bufs=2)
        nc.vector.reciprocal(
            r[0:96, :, 0], pavt[0:96, :, 64:65].rearrange("p a b -> p (a b)"))
        if h % 2 == 0:
            s["outTn"] = act.tile([96, 6, PD], BF16, name=f"oTn{b}_{h//2}",
                                  tag="oTn", bufs=2)
        oTn = s["outTn"]
        in0, in1 = broadcast_tensor_aps(pavt[0:96, :, 0:64], r[0:96, :, :])
        nc.vector.scalar_tensor_tensor(oTn[:, :, p0:p0 + 64], in0, CB, in1,
                                       OP.mult, OP.mult)
        if h % 2 == 1:
            hp = h // 2
            pt = pav.tile([PD, 6, 96], BF16, tag="av", bufs=2,
                          name=f"p_t{b}_{hp}")
            for ci, (n0, nsz) in enumerate(NCK):
                nc.tensor.transpose(pt[:, ci, 0:nsz], oTn[:, ci, :],
                                    identb[0:96, 0:96])
            oa = s["outall"]
            nc.vector.tensor_scalar_mul(
                oa[:, hp, 0:HW].rearrange("p (ci n) -> p ci n", ci=6),
                pt[:, 0:6, :], C8)

    def emit_fin(b, ots):
        s = st[b]
        oa = s["outall"]
        if "fin" not in s:
            s["fin"] = act.tile([PD, NCC, OAW], BF16, name=f"fin{b}", tag="fin",
                                bufs=2)
        fin = s["fin"]
        for ot in ots:
            p = pcv.tile([PD, 2, 512], F32, tag="cv", bufs=1,
                         name=f"p_f{b}_{ot}")
            for hi, (n0, nsz) in enumerate(FINH):
                o = p[:, hi, 0:nsz]
                for cp in range(2):
                    nc.tensor.matmul(
                        o, W["Wr8"][:, 2 * cp:2 * cp + 2, ot * PD:(ot + 1) * PD],
                        oa[:, 2 * cp:2 * cp + 2, n0:n0 + nsz],
                        start=(cp == 0), stop=(cp == 1), perf_mode=DR)
            nc.vector.tensor_scalar_mul(fin[:, ot, 0:288], p[:, 0, 0:288], CB)
            nc.vector.tensor_scalar_mul(fin[:, ot, 288:OAW], p[:, 1, 0:289], CB)
            nc.sync.dma_start(
                d["out"][b, ot * PD:(ot + 1) * PD, :], fin[:, ot, :])

    # ---- schedule: interleave AV(h-1) between head h's QK pairs ----
    def heads(b, fin_b=None):
        emit_head_qk_a(b, 0)
        emit_head_qk_b(b, 0)
        for h in range(1, NH):
            emit_head_qk_a(b, h)
            emit_head_av_a(b, h - 1)
            emit_head_qk_b(b, h)
            emit_head_av_b(b, h - 1)
            if fin_b is not None and h % 2 == 0:
                emit_fin(fin_b, [h // 2 - 1])
        emit_head_av_a(b, NH - 1)
        emit_head_av_b(b, NH - 1)
        if fin_b is not None:
            emit_fin(fin_b, [3])

    emit_loads(0)
    emit_qk_conv(0, "q")
    emit_qk_conv(0, "k")
    emit_loads(1)
    emit_vl_v(0)
    emit_cross(0)
    heads(0)
    emit_qk_conv(1, "q")
    emit_qk_conv(1, "k")
    emit_vl_v(1)
    emit_cross(1)
    heads(1)
    emit_fin(0, range(NCC))
    emit_fin(1, range(NCC))


_CACHE = {}


def _build():
    if "nc" in _CACHE:
        return _CACHE["nc"], _CACHE["out"]
    nc = bacc.Bacc("TRN2", target_bir_lowering=False, debug=False,
                   num_devices=NCORES)
    d = {
        "x8": nc.dram_tensor("x8", [BPC, C, HW], F8, kind="ExternalInput").ap(),
        "tm8": nc.dram_tensor("tm8", [BPC, C, NH], F8,
                              kind="ExternalInput").ap(),
        "out": nc.dram_tensor("out", [BPC, C, OAW], BF16,
                              kind="ExternalOutput").ap(),
        "scr_q": nc.dram_tensor("scr_q", [BPC, C, HW], F8, kind="Internal").ap(),
        "scr_k": nc.dram_tensor("scr_k", [BPC, C, HW], F8, kind="Internal").ap(),
    }
    for wn in ("Wq8", "Wk8", "Wm18", "Wv8", "Wr8"):
        d[wn] = nc.dram_tensor(wn, [C, C], F8, kind="ExternalInput").ap()
    with tile.TileContext(nc) as tc:
        with ExitStack() as ctx:
            _body(ctx, tc, d)
    nc.compile()
    _CACHE["nc"], _CACHE["out"] = nc, d["out"].tensor.name
    return nc, _CACHE["out"]


def _prep_inputs(x, t, Wk, Wq, Wt_w, Wt_b, Wm, Wv, Wr_w, Wr_b):
    f = np.float32
    f8 = mybir.dt.np(F8)
    x = np.asarray(x, f).reshape(B, C, HW)
    t = np.asarray(t, f)
    Wm1 = np.asarray(Wm, f)[:, :C]
    t_m = t @ np.asarray(Wt_w, f).T + np.asarray(Wt_b, f)
    tm_blk = np.zeros((B, C, NH), f)
    for h in range(NH):
        tm_blk[:, h * CPH:(h + 1) * CPH, h] = t_m[:, h * CPH:(h + 1) * CPH] * 32.0
    vb = (t @ np.asarray(Wm, f)[:, C:].T) @ np.asarray(Wv, f).T
    bias_host = (np.asarray(Wr_b, f)[None, :]
                 + 2.0 * (vb @ np.asarray(Wr_w, f).T))          # [B, C]
    com = {
        "Wq8": np.ascontiguousarray(np.asarray(Wq, f).T * 32).astype(f8),
        "Wk8": np.ascontiguousarray(np.asarray(Wk, f).T * 32).astype(f8),
        "Wm18": np.ascontiguousarray(Wm1.T * 32).astype(f8),
        "Wv8": np.ascontiguousarray(np.asarray(Wv, f).T * 32).astype(f8),
        "Wr8": np.ascontiguousarray(np.asarray(Wr_w, f).T * 32).astype(f8),
    }
    maps = []
    for c in range(NCORES):
        sl = slice(c * BPC, (c + 1) * BPC)
        m = dict(com)
        m["x8"] = np.ascontiguousarray(x[sl]).astype(f8)
        m["tm8"] = np.ascontiguousarray(tm_blk[sl]).astype(f8)
        maps.append(m)
    return maps, bias_host


def kernel(x, t, Wk, Wq, Wt_w, Wt_b, Wm, Wv, Wr_w, Wr_b, _trace=False):
    nc, out_name = _build()
    maps, bias_host = _prep_inputs(x, t, Wk, Wq, Wt_w, Wt_b, Wm, Wv, Wr_w, Wr_b)
    res = run_bass_kernel_spmd(nc, maps, core_ids=list(range(NCORES)),
                               trace=_trace)
    raw = np.concatenate([res.results[c][out_name].astype(np.float32)
                          for c in range(NCORES)], axis=0)   # [B, C, 577]
    c8, cb = 1.0625, 1.001953125
    dmain = 2.0 ** 12 * c8 ** 3 * cb ** 2
    dcross = 2.0 ** 10 * c8 ** 4 * cb ** 2
    out = (raw[:, :, :HW] / dmain
           + raw[:, :, HW:HW + 1] / dcross
           + bias_host[:, :, None]).astype(np.float32)
    if _trace:
        kernel.last_results = res
    return out.reshape(B, C, 24, 24)

